# revision 1
# baseline (speedup 1.0000x reference)
"""GGNN (gated graph NN) message-passing kernel for 8 Trainium2 NeuronCores.

Sharding: edge-type sharding. Core c owns edge-type block c of the adjacency
matrix (columns c*N..(c+1)*N of the [N, 2E*N] adjacency, pre-transposed on the
host) plus the node shard c for the GRU update.

Per step, on core c:
  stage1: t_c = h @ W_prop[c]                      [N, D]   (h^T streamed as lhsT)
  stage2: partial_a_c = A_cT.T @ t_c               [N, D]   (A_cT resident uint8)
  RS:     a_shard = ReduceScatter_add(partial_a)   [N/8, D] (split in 2 halves so
          the first RS overlaps the second half of stage2)
  GRU:    h_shard' = GRU(a_shard, h_shard)         (transposed layout, fp32r mm)
  AG:     h^T' = AllGather(h_shard'^T)             (fp32r)

Each core's node shard is blocks {128c..128c+127, 1024+128c..1024+128c+127}
(the blocks the two half-ReduceScatters deliver to rank c).

Numerics: matmuls in float32r (fp32 with 12-bit mantissa, full PE rate at
free-dim>=256); adjacency stored as uint8 (exact for 0/1) upconverted to fp32r
on DVE; accumulation fp32 in PSUM; elementwise GRU update in fp32.
"""
import sys
if "/opt/trn_rl_repo" not in sys.path:
    sys.path.insert(0, "/opt/trn_rl_repo")

import numpy as np
import ml_dtypes

NC_CORES = 8
N = 2048          # nodes
D = 512           # state dim
ANN = 256         # annotation dim
STEPS = 5
SH = N // NC_CORES   # 256 nodes per shard
KT = D // 128        # 4
MT = N // 128        # 16


def _q12(x):
    """Round fp32 to 12 explicit mantissa bits (fp32r grid), RNE."""
    mant, ex = np.frexp(np.asarray(x, np.float32).astype(np.float64))
    return (np.round(mant * 4096) / 4096 * np.exp2(ex)).astype(np.float32)


def build(repeats=1, ablate=()):
    import concourse.bacc as bacc
    import concourse.mybir as mybir
    import concourse.tile as tile
    from concourse.masks import make_identity

    dt = mybir.dt
    nc = bacc.Bacc()
    at_p = nc.declare_dram_parameter("at", [N, N], dt.uint8, isOutput=False)
    h0t_p = nc.declare_dram_parameter("h0t", [NC_CORES * D, SH], dt.float32r,
                                      isOutput=False)
    h0sr_p = nc.declare_dram_parameter("h0sr", [D, SH], dt.float32r, isOutput=False)
    h0s_p = nc.declare_dram_parameter("h0s", [D, SH], dt.float32, isOutput=False)
    wc_p = nc.declare_dram_parameter("wc", [D, D], dt.float32r, isOutput=False)
    gw_p = nc.declare_dram_parameter("gw", [6, D, D], dt.float32r, isOutput=False)
    bpc_p = nc.declare_dram_parameter("bpc", [1, D], dt.float32, isOutput=False)
    bz_p = nc.declare_dram_parameter("bzc", [D, 1], dt.float32, isOutput=False)
    br_p = nc.declare_dram_parameter("brc", [D, 1], dt.float32, isOutput=False)
    bh_p = nc.declare_dram_parameter("bhc", [D, 1], dt.float32, isOutput=False)
    out_p = nc.declare_dram_parameter("out", [D, SH], dt.float32, isOutput=True)
    RG = [list(range(NC_CORES))]

    from contextlib import ExitStack
    with tile.TileContext(nc) as tc, ExitStack() as stk:
        res = stk.enter_context(tc.tile_pool(name="res", bufs=1))
        p_mm = stk.enter_context(tc.tile_pool(name="pmm", bufs=8, space="PSUM"))
        p_hc = stk.enter_context(tc.tile_pool(name="phc", bufs=6))
        p_t = stk.enter_context(tc.tile_pool(name="pt", bufs=1))
        p_ar = stk.enter_context(tc.tile_pool(name="par", bufs=3))
        p_asb = stk.enter_context(tc.tile_pool(name="pasb", bufs=2))
        p_sm = stk.enter_context(tc.tile_pool(name="psm", bufs=1))
        p_h = stk.enter_context(tc.tile_pool(name="ph", bufs=2))
        dram = stk.enter_context(tc.tile_pool(name="dram", bufs=2, space="DRAM"))

        # ---- setup: constants, weights, adjacency ----
        identity = res.tile([128, 128], dt.float32, tag="identity")
        make_identity(nc, identity[:])
        ones = res.tile([1, 128], dt.float32, tag="ones")
        nc.vector.memset(ones[:], 1.0)
        bpc_t = res.tile([1, D], dt.float32, tag="bpc")
        nc.sync.dma_start(bpc_t[:], bpc_p[:])
        pb = p_mm.tile([128, D], dt.float32, tag="mm")
        nc.tensor.matmul(pb[:], ones[:], bpc_t[:], start=True, stop=True)
        bias_bcast = res.tile([128, D], dt.float32, tag="bias_bcast")
        nc.vector.tensor_copy(bias_bcast[:], pb[:])

        bias_tiles = {}
        for nm, par in (("z", bz_p), ("r", br_p), ("h", bh_p)):
            for f in range(KT):
                bt = res.tile([128, 1], dt.float32, tag=f"b{nm}{f}")
                nc.sync.dma_start(bt[:], par[f * 128:(f + 1) * 128, :])
                bias_tiles[(nm, f)] = bt

        wc_t = []
        for k in range(KT):
            w = res.tile([128, D], dt.float32r, tag=f"wc{k}")
            nc.sync.dma_start(w[:], wc_p[k * 128:(k + 1) * 128, :])
            wc_t.append(w)

        at_t = []
        for m in range(MT):
            a = res.tile([128, N], dt.uint8, tag=f"at{m}")
            nc.sync.dma_start(a[:], at_p[m * 128:(m + 1) * 128, :])
            at_t.append(a)

        # resident GRU weights (fp32r), loaded once
        gw_res = []
        for g in range(6):
            w = res.tile([128, KT, D], dt.float32r, tag=f"gwr{g}")
            nc.scalar.dma_start(w[:], gw_p[g].rearrange("(k p) f -> p k f", p=128))
            gw_res.append(w)

        for rep in range(repeats):
          # step-0 h state
          hsh_prev = []   # h^T shard, fp32r (GRU rhs)
          h32_prev = []   # h^T shard, fp32 (elementwise state)
          for k in range(KT):
            hr = p_h.tile([128, SH], dt.float32r, tag=f"hnr{k}")
            nc.sync.dma_start(hr[:], h0sr_p[k * 128:(k + 1) * 128, :])
            hsh_prev.append(hr)
            h3 = p_h.tile([128, SH], dt.float32, tag=f"h32{k}")
            nc.sync.dma_start(h3[:], h0s_p[k * 128:(k + 1) * 128, :])
            h32_prev.append(h3)

          ag_out_prev = None

          for s in range(STEPS):
             # ---- stage 1: t = h @ W_c  (+ b_c via broadcast add on cast) ----
             # shard layout: core cp owns node blocks {128cp, 1024+128cp}
             t_tiles = [None] * MT
             for mp in range(MT // 2):
                 if "s1" not in ablate:
                     hc = p_hc.tile([128, KT, 2, 128], dt.float32r, tag="hc")
                     blk = (h0t_p if s == 0 else ag_out_prev)[512 * mp:512 * (mp + 1), :]
                     nc.sync.dma_start(
                         hc[:], blk.rearrange("(k p) mj -> p k mj", p=128))
                 for mloc in range(2):
                     m = mp + 8 * mloc
                     pt = p_mm.tile([128, D], dt.float32, tag="mm")
                     if "s1" in ablate:
                         nc.tensor.matmul(pt[:], wc_t[0][:, 0:128], wc_t[1][:],
                                          start=True, stop=True)
                     else:
                         for k in range(KT):
                             nc.tensor.matmul(pt[:], hc[:, k, mloc, :], wc_t[k][:],
                                              start=(k == 0), stop=(k == KT - 1))
                     tm = p_t.tile([128, D], dt.float32r, tag=f"t{m}")
                     nc.vector.tensor_add(tm[:], pt[:], bias_bcast[:])
                     t_tiles[m] = tm

             # ---- stage 2: partial_a = A_cT.T @ t; RS per half (overlapped) ----
             rs_outs = []
             for grp in range(2):
                 rs_in = dram.tile([N // 2, D], dt.float32, tag=f"rs_in{grp}",
                                   name=f"rs_in{grp}")
                 pas = [p_mm.tile([128, D], dt.float32, tag="mm", name=f"pa{grp}_{i}")
                        for i in range(8)]
                 if "s2" in ablate:
                     for i in range(8):
                         nc.tensor.matmul(pas[i][:], t_tiles[0][:, 0:128],
                                          t_tiles[1][:], start=True, stop=True)
                 else:
                  for m in range(MT):
                     ar = p_ar.tile([128, 1024], dt.float32r, tag="ar")
                     nc.vector.tensor_copy(ar[:], at_t[m][:, grp * 1024:(grp + 1) * 1024])
                     for i in range(8):
                         nc.tensor.matmul(pas[i][:], ar[:, i * 128:(i + 1) * 128],
                                          t_tiles[m][:],
                                          start=(m == 0), stop=(m == MT - 1))
                 for i in range(8):
                     n = grp * 8 + i
                     asb = p_asb.tile([128, D], dt.float32, tag="asb")
                     if i % 2 == 0:
                         nc.scalar.copy(asb[:], pas[i][:])
                     else:
                         nc.vector.tensor_copy(asb[:], pas[i][:])
                     eng = nc.sync if i % 2 == 0 else nc.scalar
                     eng.dma_start(rs_in[i * 128:(i + 1) * 128, :], asb[:])
                 # RS of this half: core c receives node block grp*1024 + 128c
                 rs_out = dram.tile([128, D], dt.float32, tag=f"rs_out{grp}",
                                    name=f"rs_out{grp}")
                 if "cc" in ablate or "rs" in ablate:
                     nc.sync.dma_start(rs_out[:], rs_in[0:128, :])
                 else:
                     nc.gpsimd.collective_compute(
                         "ReduceScatter", mybir.AluOpType.add, replica_groups=RG,
                         ins=[rs_in[:]], outs=[rs_out[:]])
                 rs_outs.append(rs_out)

             # ---- transpose a_shard -> aT [D, SH] fp32r ----
             # r2=0 chunks (from RS1) transpose while RS2 is still in flight
             an_tiles = []
             for r2 in range(2):
                 an = p_sm.tile([128, D], dt.float32, tag=f"an{r2}")
                 nc.sync.dma_start(an[:], rs_outs[r2][:])
                 an_tiles.append(an)
             aT = []
             for kb in range(KT):
                 a_kb = p_sm.tile([128, SH], dt.float32r, tag=f"aT{kb}")
                 aT.append(a_kb)
             for r2 in range(2):
                 for kb in range(KT):
                     ptr = p_mm.tile([128, 128], dt.float32, tag="mm")
                     nc.tensor.transpose(ptr[:], an_tiles[r2][:, kb * 128:(kb + 1) * 128],
                                         identity[:])
                     nc.vector.tensor_copy(aT[kb][:, r2 * 128:(r2 + 1) * 128], ptr[:])

             # ---- GRU gates (transposed layout [D, SH]) ----
             def gate_mm(widx, uidx, rhs_u, func, bias_nm, out_dtype=dt.float32):
                 Wq, Uq = gw_res[widx], gw_res[uidx]
                 outs = []
                 for f in range(KT):
                     pg = p_mm.tile([128, SH], dt.float32, tag="mm")
                     if "gru" in ablate:
                         nc.tensor.matmul(pg[:], aT[0][:, 0:128], aT[0][:],
                                          start=True, stop=True)
                         nc.tensor.matmul(pg[:], rhs_u[0][:, 0:128], rhs_u[0][:],
                                          start=False, stop=True)
                         k = None
                     else:
                      for k in range(KT):
                         nc.tensor.matmul(pg[:], Wq[:, k, f * 128:(f + 1) * 128],
                                          aT[k][:], start=(k == 0), stop=False)
                      for k in range(KT):
                         nc.tensor.matmul(pg[:], Uq[:, k, f * 128:(f + 1) * 128],
                                          rhs_u[k][:], start=False, stop=(k == KT - 1))
                     og = p_sm.tile([128, SH], out_dtype, tag=f"g{bias_nm}{f}")
                     nc.scalar.activation(og[:], pg[:], func,
                                          bias=bias_tiles[(bias_nm, f)][:])
                     outs.append(og)
                 return outs

             import concourse.mybir as _mb
             if "gru" in ablate:
                 z_t = gate_mm(0, 1, hsh_prev, _mb.ActivationFunctionType.Sigmoid, "z")
                 r_t = gate_mm(2, 3, hsh_prev, _mb.ActivationFunctionType.Sigmoid, "r")
             else:
                 # z and r gates with both U-halves emitted first: the U-term
                 # matmuls depend only on local h and run while RS2 is in flight
                 pz = [p_mm.tile([128, SH], dt.float32, tag="mm", name=f"pz{f}")
                       for f in range(KT)]
                 pr = [p_mm.tile([128, SH], dt.float32, tag="mm", name=f"pr{f}")
                       for f in range(KT)]
                 for pg_l, uidx in ((pz, 1), (pr, 3)):
                     Uq = gw_res[uidx]
                     for f in range(KT):
                         for k in range(KT):
                             nc.tensor.matmul(pg_l[f][:],
                                              Uq[:, k, f * 128:(f + 1) * 128],
                                              hsh_prev[k][:],
                                              start=(k == 0), stop=False)
                 for pg_l, widx in ((pz, 0), (pr, 2)):
                     Wq = gw_res[widx]
                     for f in range(KT):
                         for k in range(KT):
                             nc.tensor.matmul(pg_l[f][:],
                                              Wq[:, k, f * 128:(f + 1) * 128],
                                              aT[k][:],
                                              start=False, stop=(k == KT - 1))
                 z_t, r_t = [], []
                 for outs, pg_l, nm, fn in (
                         (z_t, pz, "z", _mb.ActivationFunctionType.Sigmoid),
                         (r_t, pr, "r", _mb.ActivationFunctionType.Sigmoid)):
                     for f in range(KT):
                         og = p_sm.tile([128, SH], dt.float32, tag=f"g{nm}{f}",
                                        name=f"g{nm}{f}")
                         nc.scalar.activation(og[:], pg_l[f][:], fn,
                                              bias=bias_tiles[(nm, f)][:])
                         outs.append(og)
             rh = []
             for k in range(KT):
                 rhk = p_sm.tile([128, SH], dt.float32r, tag=f"rh{k}")
                 nc.vector.tensor_mul(rhk[:], r_t[k][:], h32_prev[k][:])
                 rh.append(rhk)
             ht_t = gate_mm(4, 5, rh, _mb.ActivationFunctionType.Tanh, "h")

             # ---- h' = h + z * (ht - h) ----
             hsh_new, h32_new = [], []
             last = (s == STEPS - 1)
             if not last:
                 ag_in = dram.tile([D, SH], dt.float16, tag="ag_in")
             for k in range(KT):
                 s1 = p_sm.tile([128, SH], dt.float32, tag="gsA")
                 nc.vector.tensor_sub(s1[:], ht_t[k][:], h32_prev[k][:])
                 s2 = p_sm.tile([128, SH], dt.float32, tag="gsB")
                 nc.vector.tensor_mul(s2[:], z_t[k][:], s1[:])
                 h3 = p_h.tile([128, SH], dt.float32, tag=f"h32{k}")
                 nc.vector.tensor_add(h3[:], h32_prev[k][:], s2[:])
                 h32_new.append(h3)
                 if last:
                     nc.sync.dma_start(out_p[k * 128:(k + 1) * 128, :], h3[:])
                 else:
                     hr = p_h.tile([128, SH], dt.float32r, tag=f"hnr{k}")
                     nc.vector.tensor_copy(hr[:], h3[:])
                     hsh_new.append(hr)
                     h16 = p_sm.tile([128, SH], dt.float16, tag=f"h16{k}",
                                     name=f"h16{k}")
                     nc.scalar.copy(h16[:], h3[:])
                     nc.sync.dma_start(ag_in[k * 128:(k + 1) * 128, :], h16[:])

             if not last:
                 ag_out = dram.tile([NC_CORES * D, SH], dt.float16, tag="ag_out",
                                    addr_space="Shared")
                 if "cc" in ablate or "ag" in ablate:
                     nc.sync.dma_start(ag_out[0:D, :], ag_in[:])
                 else:
                     nc.gpsimd.collective_compute(
                         "AllGather", mybir.AluOpType.bypass, replica_groups=RG,
                         ins=[ag_in[:]], outs=[ag_out[:]])
                 ag_out_prev = ag_out
                 hsh_prev, h32_prev = hsh_new, h32_new

    nc.finalize()
    return nc


def build2(repeats=1, ablate=()):
    """v2: stage2 as fp8 DoubleRow hi/lo cascade with t stationary.

    Layouts per step (per core, edge type c):
      stage1: t[m] = h @ W_c        t node-major [128 n, 512 d] PSUM, m=0..15
      q/r:    tq[m] = [q | r] fp8e4 [128, 1024]  (q=e4m3(t), r=e4m3(16(t-q)))
      stage2: aT[kd] += tq[m].T (x) at2[m]  via DoubleRow slots (A, A/16)
              out aT d-major [128 d(kd), 512 n] PSUM accs, 4 per kd phase
      RS:     rank-blocked d-major ReduceScatter in fp16, kd-pairs A/B
              (input block r = aT[:, nodes_r]; shard r = nodes 256r..256r+255)
      GRU:    transposed layout, gw fp32r stationary x fp16 moving
      AG:     h'^T [D, SH] fp16
    """
    import concourse.bacc as bacc
    import concourse.mybir as mybir
    import concourse.tile as tile

    dt = mybir.dt
    nc = bacc.Bacc()
    at2_p = nc.declare_dram_parameter("at2", [N, 2 * N], dt.float8e4,
                                      isOutput=False)
    h0t_p = nc.declare_dram_parameter("h0t", [NC_CORES * D, SH], dt.float16,
                                      isOutput=False)
    h0sr_p = nc.declare_dram_parameter("h0sr", [D, SH], dt.float32r,
                                       isOutput=False)
    h0s_p = nc.declare_dram_parameter("h0s", [D, SH], dt.float32, isOutput=False)
    wc_p = nc.declare_dram_parameter("wc", [D, D], dt.float16, isOutput=False)
    gw_p = nc.declare_dram_parameter("gw", [6, D, D], dt.float32r, isOutput=False)
    ba_p = nc.declare_dram_parameter("ba", [D, SH], dt.float32, isOutput=False)
    bz_p = nc.declare_dram_parameter("bzc", [D, 1], dt.float32, isOutput=False)
    br_p = nc.declare_dram_parameter("brc", [D, 1], dt.float32, isOutput=False)
    bh_p = nc.declare_dram_parameter("bhc", [D, 1], dt.float32, isOutput=False)
    out_p = nc.declare_dram_parameter("out", [D, SH], dt.float32, isOutput=True)
    RG = [list(range(NC_CORES))]

    from contextlib import ExitStack
    with tile.TileContext(nc) as tc, ExitStack() as stk:
        res = stk.enter_context(tc.tile_pool(name="res", bufs=1))
        p_mm = stk.enter_context(tc.tile_pool(name="pmm", bufs=8, space="PSUM"))
        p_hc = stk.enter_context(tc.tile_pool(name="phc", bufs=3))
        p_t = stk.enter_context(tc.tile_pool(name="pt", bufs=1))
        p_tmp = stk.enter_context(tc.tile_pool(name="ptmp", bufs=2))
        p_asb = stk.enter_context(tc.tile_pool(name="pasb", bufs=2))
        p_sm = stk.enter_context(tc.tile_pool(name="psm", bufs=1))
        p_h = stk.enter_context(tc.tile_pool(name="ph", bufs=2))
        dram = stk.enter_context(tc.tile_pool(name="dram", bufs=2, space="DRAM"))

        # ---- setup: biases, weights, adjacency pairs ----
        bias_tiles = {}
        for nm, par in (("z", bz_p), ("r", br_p), ("h", bh_p)):
            for f in range(KT):
                bt = res.tile([128, 1], dt.float32, tag=f"b{nm}{f}")
                nc.sync.dma_start(bt[:], par[f * 128:(f + 1) * 128, :])
                bias_tiles[(nm, f)] = bt

        ba_t = []
        for k in range(KT):
            b = res.tile([128, SH], dt.float32, tag=f"ba{k}")
            nc.sync.dma_start(b[:], ba_p[k * 128:(k + 1) * 128, :])
            ba_t.append(b)

        wc_t = []
        for k in range(KT):
            w = res.tile([128, D], dt.float16, tag=f"wc{k}")
            nc.sync.dma_start(w[:], wc_p[k * 128:(k + 1) * 128, :])
            wc_t.append(w)

        at_t = []
        for m in range(MT):
            a = res.tile([128, 2 * N], dt.float8e4, tag=f"at{m}")
            nc.scalar.dma_start(a[:], at2_p[m * 128:(m + 1) * 128, :])
            at_t.append(a)

        gw_res = []
        for g in range(6):
            w = res.tile([128, KT, D], dt.float32r, tag=f"gwr{g}")
            nc.scalar.dma_start(w[:], gw_p[g].rearrange("(k p) f -> p k f", p=128))
            gw_res.append(w)

        for rep in range(repeats):
          hsh_prev = []   # h^T shard fp16 (GRU U rhs)
          h32_prev = []   # h^T shard fp32 (elementwise)
          for k in range(KT):
            hr = p_h.tile([128, SH], dt.float32r, tag=f"hnr{k}")
            nc.sync.dma_start(hr[:], h0sr_p[k * 128:(k + 1) * 128, :])
            hsh_prev.append(hr)
            h3 = p_h.tile([128, SH], dt.float32, tag=f"h32{k}")
            nc.sync.dma_start(h3[:], h0s_p[k * 128:(k + 1) * 128, :])
            h32_prev.append(h3)

          ag_out_prev = None

          for s in range(STEPS):
             import concourse.mybir as _mb
             kt_u = 2 if s == 0 else KT

             # ---- stage 1 + q/r cascade: tq[m] = [fp8(t) | fp8(16(t-q))] ----
             tq = [None] * MT
             for mp in range(NC_CORES):
                 if "s1" not in ablate:
                     hc = p_hc.tile([128, KT, 2, 128], dt.float16, tag="hc")
                     blk = (h0t_p if s == 0 else ag_out_prev)[
                         D * mp:D * (mp + 1), :]
                     nc.sync.dma_start(
                         hc[:], blk.rearrange("(k p) mj -> p k mj", p=128))
                 for mloc in range(2):
                     m = 2 * mp + mloc
                     pt = p_mm.tile([128, D], dt.float32, tag="mm")
                     kt_s = 2 if s == 0 else KT
                     if "s1" in ablate:
                         nc.tensor.matmul(pt[:], wc_t[0][:, 0:128], wc_t[1][:],
                                          start=True, stop=True)
                     else:
                         for k in range(kt_s):
                             nc.tensor.matmul(pt[:], hc[:, k, mloc, :], wc_t[k][:],
                                              start=(k == 0), stop=(k == kt_s - 1))
                     tqm = p_t.tile([128, 2 * D], dt.float8e4, tag=f"tq{m}")
                     nc.scalar.copy(tqm[:, 0:D], pt[:])
                     tmp = p_tmp.tile([128, D], dt.float32, tag="tmp")
                     nc.vector.tensor_sub(tmp[:], pt[:], tqm[:, 0:D])
                     nc.scalar.activation(tqm[:, D:2 * D], tmp[:],
                                          mybir.ActivationFunctionType.Copy,
                                          scale=16.0)
                     tq[m] = tqm

             # ---- stage 2: DoubleRow kd-phases; RS per kd-pair ----
             rs_single = "rs_single" in ablate
             halves_cfg = ((0, 4),) if rs_single else ((0, 3), (1, 1))
             rs_ins, rs_outs = [], []
             for half, nk in halves_cfg:
                 ri = dram.tile([NC_CORES * nk * 128, SH], dt.float32,
                                tag=f"rs_in{half}", name=f"rs_in{half}")
                 ro = dram.tile([nk * 128, SH], dt.float32, tag=f"rs_out{half}",
                                name=f"rs_out{half}")
                 rs_ins.append(ri)
                 rs_outs.append(ro)
             for kd in range(KT):
                 accs = [p_mm.tile([128, D], dt.float32, tag="mm",
                                   name=f"acc{kd}_{q}") for q in range(4)]
                 if "s2" in ablate:
                     for q in range(4):
                         nc.tensor.matmul(accs[q][:], wc_t[0][:, 0:128],
                                          wc_t[1][:], start=True, stop=True)
                 else:
                     for m in range(MT):
                         lhs = tq[m][:].rearrange("p (o d) -> p o d", o=2)
                         rhs = at_t[m][:].rearrange("p (o n) -> p o n", o=2)
                         for q in range(4):
                             nc.tensor.matmul(
                                 accs[q][:], lhs[:, :, kd * 128:(kd + 1) * 128],
                                 rhs[:, :, q * D:(q + 1) * D],
                                 start=(m == 0), stop=(m == MT - 1),
                                 perf_mode=mybir.MatmulPerfMode.DoubleRow)
                 if rs_single:
                     half, nk, ki = 0, 4, kd
                 else:
                     half = 0 if kd < 3 else 1
                     nk = 3 if half == 0 else 1
                     ki = kd % 3
                 for q in range(4):
                     asb = p_asb.tile([128, D], dt.float32, tag="asb")
                     if q % 2 == 0:
                         nc.scalar.copy(asb[:], accs[q][:])
                     else:
                         nc.vector.tensor_copy(asb[:], accs[q][:])
                     eng = nc.sync if q % 2 == 0 else nc.scalar
                     for rr in range(2):
                         row0 = (2 * q + rr) * nk * 128 + ki * 128
                         eng.dma_start(rs_ins[half][row0:row0 + 128, :],
                                       asb[:, rr * SH:(rr + 1) * SH])
                 if (kd == 3) if rs_single else (kd in (2, 3)):
                     if "cc" in ablate or "rs" in ablate:
                         nc.sync.dma_start(rs_outs[half][:],
                                           rs_ins[half][0:nk * 128, :])
                     else:
                         nc.gpsimd.collective_compute(
                             "ReduceScatter", mybir.AluOpType.add,
                             replica_groups=RG,
                             ins=[rs_ins[half][:]], outs=[rs_outs[half][:]])

             # ---- GRU ----
             # z/r U-parts (local h -> overlap RS latency)
             pz = [p_mm.tile([128, SH], dt.float32, tag="mm", name=f"pz{f}")
                   for f in range(KT)]
             pr = [p_mm.tile([128, SH], dt.float32, tag="mm", name=f"pr{f}")
                   for f in range(KT)]
             if "gru" in ablate:
                 for f in range(KT):
                     nc.tensor.matmul(pz[f][:], wc_t[0][:, 0:128],
                                      wc_t[1][:, 0:SH], start=True, stop=True)
                     nc.tensor.matmul(pr[f][:], wc_t[0][:, 0:128],
                                      wc_t[1][:, 0:SH], start=True, stop=True)
             else:
                 for pg_l, uidx in ((pz, 1), (pr, 3)):
                     Uq = gw_res[uidx]
                     for f in range(KT):
                         for k in range(kt_u):
                             nc.tensor.matmul(pg_l[f][:],
                                              Uq[:, k, f * 128:(f + 1) * 128],
                                              hsh_prev[k][:],
                                              start=(k == 0), stop=False)

             # aT tiles: RS outputs + bias_a, fp16
             aT = []
             for k in range(KT):
                 an = p_sm.tile([128, SH], dt.float32, tag=f"an{k}")
                 src_half = 0 if (rs_single or k < 3) else 1
                 r0 = (k if rs_single else (k % 3)) * 128
                 nc.sync.dma_start(an[:], rs_outs[src_half][r0:r0 + 128, :])
                 a_k = p_sm.tile([128, SH], dt.float32r, tag=f"aT{k}")
                 nc.vector.tensor_add(a_k[:], an[:], ba_t[k][:])
                 aT.append(a_k)

             if "gru" not in ablate:
                 # W-parts k-outer: k<3 consume RS-A, k=3 consumes RS-B
                 for k in range(KT):
                     for pg_l, widx in ((pz, 0), (pr, 2)):
                         Wq = gw_res[widx]
                         for f in range(KT):
                             nc.tensor.matmul(pg_l[f][:],
                                              Wq[:, k, f * 128:(f + 1) * 128],
                                              aT[k][:],
                                              start=False, stop=(k == KT - 1))
             z_t, r_t = [], []
             for outs, pg_l, nm, fn in (
                     (z_t, pz, "z", _mb.ActivationFunctionType.Sigmoid),
                     (r_t, pr, "r", _mb.ActivationFunctionType.Sigmoid)):
                 for f in range(KT):
                     og = p_sm.tile([128, SH], dt.float32, tag=f"g{nm}{f}",
                                    name=f"g{nm}{f}")
                     nc.scalar.activation(og[:], pg_l[f][:], fn,
                                          bias=bias_tiles[(nm, f)][:])
                     outs.append(og)
             rh = []
             for k in range(KT):
                 rhk = p_sm.tile([128, SH], dt.float32r, tag=f"rh{k}")
                 nc.vector.tensor_mul(rhk[:], r_t[k][:], h32_prev[k][:])
                 rh.append(rhk)
             ht_t = []
             ph = [p_mm.tile([128, SH], dt.float32, tag="mm", name=f"ph{f}")
                   for f in range(KT)]
             if "gru" in ablate:
                 for f in range(KT):
                     nc.tensor.matmul(ph[f][:], wc_t[0][:, 0:128],
                                      wc_t[1][:, 0:SH], start=True, stop=True)
             else:
                 Wq, Uq = gw_res[4], gw_res[5]
                 for f in range(KT):
                     for k in range(kt_u):
                         nc.tensor.matmul(ph[f][:],
                                          Uq[:, k, f * 128:(f + 1) * 128],
                                          rh[k][:], start=(k == 0), stop=False)
                     for k in range(KT):
                         nc.tensor.matmul(ph[f][:],
                                          Wq[:, k, f * 128:(f + 1) * 128],
                                          aT[k][:], start=False,
                                          stop=(k == KT - 1))
             for f in range(KT):
                 og = p_sm.tile([128, SH], dt.float32, tag=f"gh{f}",
                                name=f"gh{f}")
                 nc.scalar.activation(og[:], ph[f][:],
                                      _mb.ActivationFunctionType.Tanh,
                                      bias=bias_tiles[("h", f)][:])
                 ht_t.append(og)

             # ---- h' = h + z * (ht - h) ----
             hsh_new, h32_new = [], []
             last = (s == STEPS - 1)
             if not last:
                 ag_in = dram.tile([D, SH], dt.float16, tag="ag_in")
             for k in range(KT):
                 s1 = p_sm.tile([128, SH], dt.float32, tag="gsA")
                 nc.vector.tensor_sub(s1[:], ht_t[k][:], h32_prev[k][:])
                 s2 = p_sm.tile([128, SH], dt.float32, tag="gsB")
                 nc.vector.tensor_mul(s2[:], z_t[k][:], s1[:])
                 h3 = p_h.tile([128, SH], dt.float32, tag=f"h32{k}")
                 nc.vector.tensor_add(h3[:], h32_prev[k][:], s2[:])
                 h32_new.append(h3)
                 if last:
                     nc.sync.dma_start(out_p[k * 128:(k + 1) * 128, :], h3[:])
                 else:
                     hr = p_h.tile([128, SH], dt.float32r, tag=f"hnr{k}")
                     nc.vector.tensor_copy(hr[:], h3[:])
                     hsh_new.append(hr)
                     h16 = p_sm.tile([128, SH], dt.float16, tag=f"h16{k}",
                                     name=f"h16{k}")
                     nc.scalar.copy(h16[:], h3[:])
                     nc.sync.dma_start(ag_in[k * 128:(k + 1) * 128, :], h16[:])

             if not last:
                 ag_out = dram.tile([NC_CORES * D, SH], dt.float16, tag="ag_out",
                                    addr_space="Shared")
                 if "cc" in ablate or "ag" in ablate:
                     nc.sync.dma_start(ag_out[0:D, :], ag_in[:])
                 else:
                     nc.gpsimd.collective_compute(
                         "AllGather", mybir.AluOpType.bypass, replica_groups=RG,
                         ins=[ag_in[:]], outs=[ag_out[:]])
                 ag_out_prev = ag_out
                 hsh_prev, h32_prev = hsh_new, h32_new

    nc.finalize()
    return nc


def prepare_in_maps2(adjacency, annotations, W_prop, b_prop, Wz, Uz, bz,
                     Wr, Ur, br, Wh, Uh, bh):
    A = np.asarray(adjacency, np.float32)
    ann = np.asarray(annotations, np.float32)
    W_prop = np.asarray(W_prop, np.float32)
    b_prop = np.asarray(b_prop, np.float32)
    gw_all = _q12(np.stack([np.asarray(x, np.float32)
                            for x in (Wz, Uz, Wr, Ur, Wh, Uh)]))
    bz = np.asarray(bz, np.float32).reshape(D, 1)
    br = np.asarray(br, np.float32).reshape(D, 1)
    bh = np.asarray(bh, np.float32).reshape(D, 1)

    h0 = np.zeros((N, D), np.float32)
    h0[:, :ann.shape[1]] = ann
    h0t = np.ascontiguousarray(h0.T)           # [D, N] fp32
    h0t16 = h0t.astype(np.float16)
    A_T = np.ascontiguousarray(A.T)            # [2E*N, N]

    # bias_a[n, :] = sum_e deg_e(n) * b_e ; transposed shard [D, SH]
    deg = A.reshape(N, 2 * E_TYPES, N).sum(axis=2)      # [N, 2E]
    bias_a = deg @ b_prop                               # [N, D]
    bias_aT = np.ascontiguousarray(bias_a.T)            # [D, N]

    # contiguous shards: core c owns nodes 256c..256c+255
    h0t_ag = np.ascontiguousarray(h0t16)  # same layout: [D, N] -> per-core
    # AG layout: [NC*D, SH]: block mp = core mp's [D, SH]
    h0t_ag = np.concatenate(
        [h0t16[:, c * SH:(c + 1) * SH] for c in range(NC_CORES)], axis=0)

    in_maps = []
    for c in range(NC_CORES):
        at_c = A_T[c * N:(c + 1) * N, :]               # [N j, N n] 0/1
        at8 = at_c.astype(ml_dtypes.float8_e4m3)
        at8_lo = (at_c / 16.0).astype(ml_dtypes.float8_e4m3)
        at2 = np.concatenate([at8, at8_lo], axis=1)    # [N, 2N] slot-major
        in_maps.append({
            "at2": np.ascontiguousarray(at2),
            "h0t": np.ascontiguousarray(h0t_ag),
            "h0sr": _q12(np.ascontiguousarray(h0t[:, c * SH:(c + 1) * SH])),
            "h0s": np.ascontiguousarray(h0t[:, c * SH:(c + 1) * SH]),
            "wc": W_prop[c].astype(np.float16),
            "gw": gw_all,
            "ba": np.ascontiguousarray(bias_aT[:, c * SH:(c + 1) * SH]),
            "bzc": bz, "brc": br, "bhc": bh,
        })
    return in_maps


E_TYPES = 4
_BUILT = None
TRACE = False
V2 = True
LAST_RESULT = None


_BUILT_R = {}


def _get_built(repeats=1, ablate=()):
    global _BUILT
    bf = build2 if V2 else build
    key = (V2, repeats, tuple(ablate))
    if key != (True, 1, ()):
        if key not in _BUILT_R:
            _BUILT_R[key] = bf(repeats, ablate)
        return _BUILT_R[key]
    if _BUILT is None:
        _BUILT = bf()
    return _BUILT


def prepare_in_maps(adjacency, annotations, W_prop, b_prop, Wz, Uz, bz,
                    Wr, Ur, br, Wh, Uh, bh):
    A = np.asarray(adjacency, np.float32)
    ann = np.asarray(annotations, np.float32)
    W_prop = np.asarray(W_prop, np.float32)
    b_prop = np.asarray(b_prop, np.float32)
    gw_all = _q12(np.stack([np.asarray(x, np.float32)
                            for x in (Wz, Uz, Wr, Ur, Wh, Uh)]))
    bz = np.asarray(bz, np.float32).reshape(D, 1)
    br = np.asarray(br, np.float32).reshape(D, 1)
    bh = np.asarray(bh, np.float32).reshape(D, 1)

    h0 = np.zeros((N, D), np.float32)
    h0[:, :ann.shape[1]] = ann
    h0t = np.ascontiguousarray(h0.T)           # [D, N] fp32
    h0t_r = _q12(h0t)
    A_T = np.ascontiguousarray(A.T)            # [2E*N, N]

    # shard layout: core c owns node blocks {128c..128c+127, 1024+128c..+127}
    shard_cols = [np.r_[128 * c:128 * c + 128, 1024 + 128 * c:1024 + 128 * c + 128]
                  for c in range(NC_CORES)]
    h0t_ag = np.ascontiguousarray(np.concatenate(
        [h0t_r[:, shard_cols[c]] for c in range(NC_CORES)], axis=0))

    in_maps = []
    for c in range(NC_CORES):
        in_maps.append({
            "at": np.ascontiguousarray(
                A_T[c * N:(c + 1) * N, :]).astype(np.uint8),
            "h0t": h0t_ag,
            "h0sr": np.ascontiguousarray(h0t_r[:, shard_cols[c]]),
            "h0s": np.ascontiguousarray(h0t[:, shard_cols[c]]),
            "wc": _q12(W_prop[c]),
            "gw": gw_all,
            "bpc": np.ascontiguousarray(b_prop[c].reshape(1, D)),
            "bzc": bz, "brc": br, "bhc": bh,
        })

    return in_maps


def kernel(**inputs):
    from concourse.bass_utils import run_bass_kernel_spmd

    prep = prepare_in_maps2 if V2 else prepare_in_maps
    in_maps = prep(
        **{k: inputs[k] for k in ("adjacency", "annotations", "W_prop", "b_prop",
                                  "Wz", "Uz", "bz", "Wr", "Ur", "br",
                                  "Wh", "Uh", "bh")})
    nc = _get_built()
    res = run_bass_kernel_spmd(nc, in_maps, list(range(NC_CORES)), trace=TRACE)
    global LAST_RESULT
    LAST_RESULT = res
    h = np.empty((N, D), np.float32)
    for c in range(NC_CORES):
        sh = res.results[c]["out"].T           # [SH, D] rows in shard order
        if V2:
            h[SH * c:SH * (c + 1)] = sh
        else:
            h[128 * c:128 * c + 128] = sh[:128]
            h[1024 + 128 * c:1024 + 128 * c + 128] = sh[128:]
    return h



# revision 14
# speedup vs baseline: 1.4562x; 1.4562x over previous
"""GGNN (gated graph NN) message-passing kernel for 8 Trainium2 NeuronCores.

v3 sharding: contract-dim (j) sharding. Core c owns node shard
[256c, 256c+256). Per step, on core c:
  stage1: t_e[shard_c] = h_shard @ (16 W_e) for ALL 8 edge types (fp32r,
          k-inner/e-inner loop so each h-slice LDWEIGHTS feeds 8 matmuls)
  quant:  tq = fp8e4m3 hi/lo pairs of t (residual exactness via the x16
          scale baked into W_prop on the host; gate weights carry /16)
  stage2: partial aT[d, n] = sum_j tq.T (x) A_sel  -- fp8 DoubleRow with
          both slots carrying REAL contraction (j-blocks m and m+8 slot-
          paired; separate hi and lo passes) so each 256-col weight load
          feeds 4 matmuls of 512 free dim
  RS:     four per-kd-phase fp16 ReduceScatters (rank-blocked [8*128, 256])
          that pipeline behind the following phase's matmuls
  GRU:    U-part matmuls (local h, fp32r) issue before the RS completes;
          W-parts consume aT k-blocks as each RS lands; h'^T stays local
          in d-major fp32r -- NO AllGather anywhere.

Numerics (model rel err 8.5e-3, HW 1.33e-2 vs 2e-2 gate): W_prop/GRU
weights/h/aT all fp32r (12-bit mantissa); adjacency exact in fp8 (0/1);
t carried as fp8 hi+lo ~= 14 bits; RS wire fp16.
"""
import sys
if "/opt/trn_rl_repo" not in sys.path:
    sys.path.insert(0, "/opt/trn_rl_repo")

import numpy as np
import ml_dtypes

NC_CORES = 8
N = 2048          # nodes
D = 512           # state dim
ANN = 256         # annotation dim
STEPS = 5
SH = N // NC_CORES   # 256 nodes per shard
KT = D // 128        # 4
MT = N // 128        # 16


def _q12(x):
    """Round fp32 to 12 explicit mantissa bits (fp32r grid), RNE."""
    mant, ex = np.frexp(np.asarray(x, np.float32).astype(np.float64))
    return (np.round(mant * 4096) / 4096 * np.exp2(ex)).astype(np.float32)


def build(repeats=1, ablate=()):
    import concourse.bacc as bacc
    import concourse.mybir as mybir
    import concourse.tile as tile
    from concourse.masks import make_identity

    dt = mybir.dt
    nc = bacc.Bacc()
    at_p = nc.declare_dram_parameter("at", [N, N], dt.uint8, isOutput=False)
    h0t_p = nc.declare_dram_parameter("h0t", [NC_CORES * D, SH], dt.float32r,
                                      isOutput=False)
    h0sr_p = nc.declare_dram_parameter("h0sr", [D, SH], dt.float32r, isOutput=False)
    h0s_p = nc.declare_dram_parameter("h0s", [D, SH], dt.float32, isOutput=False)
    wc_p = nc.declare_dram_parameter("wc", [D, D], dt.float32r, isOutput=False)
    gw_p = nc.declare_dram_parameter("gw", [6, D, D], dt.float32r, isOutput=False)
    bpc_p = nc.declare_dram_parameter("bpc", [1, D], dt.float32, isOutput=False)
    bz_p = nc.declare_dram_parameter("bzc", [D, 1], dt.float32, isOutput=False)
    br_p = nc.declare_dram_parameter("brc", [D, 1], dt.float32, isOutput=False)
    bh_p = nc.declare_dram_parameter("bhc", [D, 1], dt.float32, isOutput=False)
    out_p = nc.declare_dram_parameter("out", [D, SH], dt.float32, isOutput=True)
    RG = [list(range(NC_CORES))]

    from contextlib import ExitStack
    with tile.TileContext(nc) as tc, ExitStack() as stk:
        res = stk.enter_context(tc.tile_pool(name="res", bufs=1))
        p_mm = stk.enter_context(tc.tile_pool(name="pmm", bufs=8, space="PSUM"))
        p_hc = stk.enter_context(tc.tile_pool(name="phc", bufs=6))
        p_t = stk.enter_context(tc.tile_pool(name="pt", bufs=1))
        p_ar = stk.enter_context(tc.tile_pool(name="par", bufs=3))
        p_asb = stk.enter_context(tc.tile_pool(name="pasb", bufs=2))
        p_sm = stk.enter_context(tc.tile_pool(name="psm", bufs=1))
        p_h = stk.enter_context(tc.tile_pool(name="ph", bufs=2))
        dram = stk.enter_context(tc.tile_pool(name="dram", bufs=2, space="DRAM"))

        # ---- setup: constants, weights, adjacency ----
        identity = res.tile([128, 128], dt.float32, tag="identity")
        make_identity(nc, identity[:])
        ones = res.tile([1, 128], dt.float32, tag="ones")
        nc.vector.memset(ones[:], 1.0)
        bpc_t = res.tile([1, D], dt.float32, tag="bpc")
        nc.sync.dma_start(bpc_t[:], bpc_p[:])
        pb = p_mm.tile([128, D], dt.float32, tag="mm")
        nc.tensor.matmul(pb[:], ones[:], bpc_t[:], start=True, stop=True)
        bias_bcast = res.tile([128, D], dt.float32, tag="bias_bcast")
        nc.vector.tensor_copy(bias_bcast[:], pb[:])

        bias_tiles = {}
        for nm, par in (("z", bz_p), ("r", br_p), ("h", bh_p)):
            for f in range(KT):
                bt = res.tile([128, 1], dt.float32, tag=f"b{nm}{f}")
                nc.sync.dma_start(bt[:], par[f * 128:(f + 1) * 128, :])
                bias_tiles[(nm, f)] = bt

        wc_t = []
        for k in range(KT):
            w = res.tile([128, D], dt.float32r, tag=f"wc{k}")
            nc.sync.dma_start(w[:], wc_p[k * 128:(k + 1) * 128, :])
            wc_t.append(w)

        at_t = []
        for m in range(MT):
            a = res.tile([128, N], dt.uint8, tag=f"at{m}")
            nc.sync.dma_start(a[:], at_p[m * 128:(m + 1) * 128, :])
            at_t.append(a)

        # resident GRU weights (fp32r), loaded once
        gw_res = []
        for g in range(6):
            w = res.tile([128, KT, D], dt.float32r, tag=f"gwr{g}")
            nc.scalar.dma_start(w[:], gw_p[g].rearrange("(k p) f -> p k f", p=128))
            gw_res.append(w)

        for rep in range(repeats):
          # step-0 h state
          hsh_prev = []   # h^T shard, fp32r (GRU rhs)
          h32_prev = []   # h^T shard, fp32 (elementwise state)
          for k in range(KT):
            hr = p_h.tile([128, SH], dt.float32r, tag=f"hnr{k}")
            nc.sync.dma_start(hr[:], h0sr_p[k * 128:(k + 1) * 128, :])
            hsh_prev.append(hr)
            h3 = p_h.tile([128, SH], dt.float32, tag=f"h32{k}")
            nc.sync.dma_start(h3[:], h0s_p[k * 128:(k + 1) * 128, :])
            h32_prev.append(h3)

          ag_out_prev = None

          for s in range(STEPS):
             # ---- stage 1: t = h @ W_c  (+ b_c via broadcast add on cast) ----
             # shard layout: core cp owns node blocks {128cp, 1024+128cp}
             t_tiles = [None] * MT
             for mp in range(MT // 2):
                 if "s1" not in ablate:
                     hc = p_hc.tile([128, KT, 2, 128], dt.float32r, tag="hc")
                     blk = (h0t_p if s == 0 else ag_out_prev)[512 * mp:512 * (mp + 1), :]
                     nc.sync.dma_start(
                         hc[:], blk.rearrange("(k p) mj -> p k mj", p=128))
                 for mloc in range(2):
                     m = mp + 8 * mloc
                     pt = p_mm.tile([128, D], dt.float32, tag="mm")
                     if "s1" in ablate:
                         nc.tensor.matmul(pt[:], wc_t[0][:, 0:128], wc_t[1][:],
                                          start=True, stop=True)
                     else:
                         for k in range(KT):
                             nc.tensor.matmul(pt[:], hc[:, k, mloc, :], wc_t[k][:],
                                              start=(k == 0), stop=(k == KT - 1))
                     tm = p_t.tile([128, D], dt.float32r, tag=f"t{m}")
                     nc.vector.tensor_add(tm[:], pt[:], bias_bcast[:])
                     t_tiles[m] = tm

             # ---- stage 2: partial_a = A_cT.T @ t; RS per half (overlapped) ----
             rs_outs = []
             for grp in range(2):
                 rs_in = dram.tile([N // 2, D], dt.float32, tag=f"rs_in{grp}",
                                   name=f"rs_in{grp}")
                 pas = [p_mm.tile([128, D], dt.float32, tag="mm", name=f"pa{grp}_{i}")
                        for i in range(8)]
                 if "s2" in ablate:
                     for i in range(8):
                         nc.tensor.matmul(pas[i][:], t_tiles[0][:, 0:128],
                                          t_tiles[1][:], start=True, stop=True)
                 else:
                  for m in range(MT):
                     ar = p_ar.tile([128, 1024], dt.float32r, tag="ar")
                     nc.vector.tensor_copy(ar[:], at_t[m][:, grp * 1024:(grp + 1) * 1024])
                     for i in range(8):
                         nc.tensor.matmul(pas[i][:], ar[:, i * 128:(i + 1) * 128],
                                          t_tiles[m][:],
                                          start=(m == 0), stop=(m == MT - 1))
                 for i in range(8):
                     n = grp * 8 + i
                     asb = p_asb.tile([128, D], dt.float32, tag="asb")
                     if i % 2 == 0:
                         nc.scalar.copy(asb[:], pas[i][:])
                     else:
                         nc.vector.tensor_copy(asb[:], pas[i][:])
                     eng = nc.sync if i % 2 == 0 else nc.scalar
                     eng.dma_start(rs_in[i * 128:(i + 1) * 128, :], asb[:])
                 # RS of this half: core c receives node block grp*1024 + 128c
                 rs_out = dram.tile([128, D], dt.float32, tag=f"rs_out{grp}",
                                    name=f"rs_out{grp}")
                 if "cc" in ablate or "rs" in ablate:
                     nc.sync.dma_start(rs_out[:], rs_in[0:128, :])
                 else:
                     nc.gpsimd.collective_compute(
                         "ReduceScatter", mybir.AluOpType.add, replica_groups=RG,
                         ins=[rs_in[:]], outs=[rs_out[:]])
                 rs_outs.append(rs_out)

             # ---- transpose a_shard -> aT [D, SH] fp32r ----
             # r2=0 chunks (from RS1) transpose while RS2 is still in flight
             an_tiles = []
             for r2 in range(2):
                 an = p_sm.tile([128, D], dt.float32, tag=f"an{r2}")
                 nc.sync.dma_start(an[:], rs_outs[r2][:])
                 an_tiles.append(an)
             aT = []
             for kb in range(KT):
                 a_kb = p_sm.tile([128, SH], dt.float32r, tag=f"aT{kb}")
                 aT.append(a_kb)
             for r2 in range(2):
                 for kb in range(KT):
                     ptr = p_mm.tile([128, 128], dt.float32, tag="mm")
                     nc.tensor.transpose(ptr[:], an_tiles[r2][:, kb * 128:(kb + 1) * 128],
                                         identity[:])
                     nc.vector.tensor_copy(aT[kb][:, r2 * 128:(r2 + 1) * 128], ptr[:])

             # ---- GRU gates (transposed layout [D, SH]) ----
             def gate_mm(widx, uidx, rhs_u, func, bias_nm, out_dtype=dt.float32):
                 Wq, Uq = gw_res[widx], gw_res[uidx]
                 outs = []
                 for f in range(KT):
                     pg = p_mm.tile([128, SH], dt.float32, tag="mm")
                     if "gru" in ablate:
                         nc.tensor.matmul(pg[:], aT[0][:, 0:128], aT[0][:],
                                          start=True, stop=True)
                         nc.tensor.matmul(pg[:], rhs_u[0][:, 0:128], rhs_u[0][:],
                                          start=False, stop=True)
                         k = None
                     else:
                      for k in range(KT):
                         nc.tensor.matmul(pg[:], Wq[:, k, f * 128:(f + 1) * 128],
                                          aT[k][:], start=(k == 0), stop=False)
                      for k in range(KT):
                         nc.tensor.matmul(pg[:], Uq[:, k, f * 128:(f + 1) * 128],
                                          rhs_u[k][:], start=False, stop=(k == KT - 1))
                     og = p_sm.tile([128, SH], out_dtype, tag=f"g{bias_nm}{f}")
                     nc.scalar.activation(og[:], pg[:], func,
                                          bias=bias_tiles[(bias_nm, f)][:])
                     outs.append(og)
                 return outs

             import concourse.mybir as _mb
             if "gru" in ablate:
                 z_t = gate_mm(0, 1, hsh_prev, _mb.ActivationFunctionType.Sigmoid, "z")
                 r_t = gate_mm(2, 3, hsh_prev, _mb.ActivationFunctionType.Sigmoid, "r")
             else:
                 # z and r gates with both U-halves emitted first: the U-term
                 # matmuls depend only on local h and run while RS2 is in flight
                 pz = [p_mm.tile([128, SH], dt.float32, tag="mm", name=f"pz{f}")
                       for f in range(KT)]
                 pr = [p_mm.tile([128, SH], dt.float32, tag="mm", name=f"pr{f}")
                       for f in range(KT)]
                 for pg_l, uidx in ((pz, 1), (pr, 3)):
                     Uq = gw_res[uidx]
                     for f in range(KT):
                         for k in range(KT):
                             nc.tensor.matmul(pg_l[f][:],
                                              Uq[:, k, f * 128:(f + 1) * 128],
                                              hsh_prev[k][:],
                                              start=(k == 0), stop=False)
                 for pg_l, widx in ((pz, 0), (pr, 2)):
                     Wq = gw_res[widx]
                     for f in range(KT):
                         for k in range(KT):
                             nc.tensor.matmul(pg_l[f][:],
                                              Wq[:, k, f * 128:(f + 1) * 128],
                                              aT[k][:],
                                              start=False, stop=(k == KT - 1))
                 z_t, r_t = [], []
                 for outs, pg_l, nm, fn in (
                         (z_t, pz, "z", _mb.ActivationFunctionType.Sigmoid),
                         (r_t, pr, "r", _mb.ActivationFunctionType.Sigmoid)):
                     for f in range(KT):
                         og = p_sm.tile([128, SH], dt.float32, tag=f"g{nm}{f}",
                                        name=f"g{nm}{f}")
                         nc.scalar.activation(og[:], pg_l[f][:], fn,
                                              bias=bias_tiles[(nm, f)][:])
                         outs.append(og)
             rh = []
             for k in range(KT):
                 rhk = p_sm.tile([128, SH], dt.float32r, tag=f"rh{k}")
                 nc.vector.tensor_mul(rhk[:], r_t[k][:], h32_prev[k][:])
                 rh.append(rhk)
             ht_t = gate_mm(4, 5, rh, _mb.ActivationFunctionType.Tanh, "h")

             # ---- h' = h + z * (ht - h) ----
             hsh_new, h32_new = [], []
             last = (s == STEPS - 1)
             if not last:
                 ag_in = dram.tile([D, SH], dt.float16, tag="ag_in")
             for k in range(KT):
                 s1 = p_sm.tile([128, SH], dt.float32, tag="gsA")
                 nc.vector.tensor_sub(s1[:], ht_t[k][:], h32_prev[k][:])
                 s2 = p_sm.tile([128, SH], dt.float32, tag="gsB")
                 nc.vector.tensor_mul(s2[:], z_t[k][:], s1[:])
                 h3 = p_h.tile([128, SH], dt.float32, tag=f"h32{k}")
                 nc.vector.tensor_add(h3[:], h32_prev[k][:], s2[:])
                 h32_new.append(h3)
                 if last:
                     nc.sync.dma_start(out_p[k * 128:(k + 1) * 128, :], h3[:])
                 else:
                     hr = p_h.tile([128, SH], dt.float32r, tag=f"hnr{k}")
                     nc.vector.tensor_copy(hr[:], h3[:])
                     hsh_new.append(hr)
                     h16 = p_sm.tile([128, SH], dt.float16, tag=f"h16{k}",
                                     name=f"h16{k}")
                     nc.scalar.copy(h16[:], h3[:])
                     nc.sync.dma_start(ag_in[k * 128:(k + 1) * 128, :], h16[:])

             if not last:
                 ag_out = dram.tile([NC_CORES * D, SH], dt.float16, tag="ag_out",
                                    addr_space="Shared")
                 if "cc" in ablate or "ag" in ablate:
                     nc.sync.dma_start(ag_out[0:D, :], ag_in[:])
                 else:
                     nc.gpsimd.collective_compute(
                         "AllGather", mybir.AluOpType.bypass, replica_groups=RG,
                         ins=[ag_in[:]], outs=[ag_out[:]])
                 ag_out_prev = ag_out
                 hsh_prev, h32_prev = hsh_new, h32_new

    nc.finalize()
    return nc


def build2(repeats=1, ablate=()):
    """v2: stage2 as fp8 DoubleRow hi/lo cascade with t stationary.

    Layouts per step (per core, edge type c):
      stage1: t[m] = h @ W_c        t node-major [128 n, 512 d] PSUM, m=0..15
      q/r:    tq[m] = [q | r] fp8e4 [128, 1024]  (q=e4m3(t), r=e4m3(16(t-q)))
      stage2: aT[kd] += tq[m].T (x) at2[m]  via DoubleRow slots (A, A/16)
              out aT d-major [128 d(kd), 512 n] PSUM accs, 4 per kd phase
      RS:     rank-blocked d-major ReduceScatter in fp16, kd-pairs A/B
              (input block r = aT[:, nodes_r]; shard r = nodes 256r..256r+255)
      GRU:    transposed layout, gw fp32r stationary x fp16 moving
      AG:     h'^T [D, SH] fp16
    """
    import concourse.bacc as bacc
    import concourse.mybir as mybir
    import concourse.tile as tile

    dt = mybir.dt
    nc = bacc.Bacc()
    at2_p = nc.declare_dram_parameter("at2", [N, 2 * N], dt.float8e4,
                                      isOutput=False)
    h0t_p = nc.declare_dram_parameter("h0t", [NC_CORES * D, SH], dt.float16,
                                      isOutput=False)
    h0sr_p = nc.declare_dram_parameter("h0sr", [D, SH], dt.float32r,
                                       isOutput=False)
    h0s_p = nc.declare_dram_parameter("h0s", [D, SH], dt.float32, isOutput=False)
    wc_p = nc.declare_dram_parameter("wc", [D, D], dt.float16, isOutput=False)
    gw_p = nc.declare_dram_parameter("gw", [6, D, D], dt.float32r, isOutput=False)
    ba_p = nc.declare_dram_parameter("ba", [D, SH], dt.float32, isOutput=False)
    bz_p = nc.declare_dram_parameter("bzc", [D, 1], dt.float32, isOutput=False)
    br_p = nc.declare_dram_parameter("brc", [D, 1], dt.float32, isOutput=False)
    bh_p = nc.declare_dram_parameter("bhc", [D, 1], dt.float32, isOutput=False)
    out_p = nc.declare_dram_parameter("out", [D, SH], dt.float32, isOutput=True)
    RG = [list(range(NC_CORES))]

    from contextlib import ExitStack
    with tile.TileContext(nc) as tc, ExitStack() as stk:
        res = stk.enter_context(tc.tile_pool(name="res", bufs=1))
        p_mm = stk.enter_context(tc.tile_pool(name="pmm", bufs=8, space="PSUM"))
        p_hc = stk.enter_context(tc.tile_pool(name="phc", bufs=3))
        p_t = stk.enter_context(tc.tile_pool(name="pt", bufs=1))
        p_tmp = stk.enter_context(tc.tile_pool(name="ptmp", bufs=2))
        p_asb = stk.enter_context(tc.tile_pool(name="pasb", bufs=2))
        p_sm = stk.enter_context(tc.tile_pool(name="psm", bufs=1))
        p_h = stk.enter_context(tc.tile_pool(name="ph", bufs=2))
        dram = stk.enter_context(tc.tile_pool(name="dram", bufs=2, space="DRAM"))

        # ---- setup: biases, weights, adjacency pairs ----
        bias_tiles = {}
        for nm, par in (("z", bz_p), ("r", br_p), ("h", bh_p)):
            for f in range(KT):
                bt = res.tile([128, 1], dt.float32, tag=f"b{nm}{f}")
                nc.sync.dma_start(bt[:], par[f * 128:(f + 1) * 128, :])
                bias_tiles[(nm, f)] = bt

        ba_t = []
        for k in range(KT):
            b = res.tile([128, SH], dt.float32, tag=f"ba{k}")
            nc.sync.dma_start(b[:], ba_p[k * 128:(k + 1) * 128, :])
            ba_t.append(b)

        wc_t = []
        for k in range(KT):
            w = res.tile([128, D], dt.float16, tag=f"wc{k}")
            nc.sync.dma_start(w[:], wc_p[k * 128:(k + 1) * 128, :])
            wc_t.append(w)

        at_t = []
        for m in range(MT):
            a = res.tile([128, 2 * N], dt.float8e4, tag=f"at{m}")
            nc.scalar.dma_start(a[:], at2_p[m * 128:(m + 1) * 128, :])
            at_t.append(a)

        gw_res = []
        for g in range(6):
            w = res.tile([128, KT, D], dt.float32r, tag=f"gwr{g}")
            nc.scalar.dma_start(w[:], gw_p[g].rearrange("(k p) f -> p k f", p=128))
            gw_res.append(w)

        for rep in range(repeats):
          hsh_prev = []   # h^T shard fp16 (GRU U rhs)
          h32_prev = []   # h^T shard fp32 (elementwise)
          for k in range(KT):
            hr = p_h.tile([128, SH], dt.float32r, tag=f"hnr{k}")
            nc.sync.dma_start(hr[:], h0sr_p[k * 128:(k + 1) * 128, :])
            hsh_prev.append(hr)
            h3 = p_h.tile([128, SH], dt.float32, tag=f"h32{k}")
            nc.sync.dma_start(h3[:], h0s_p[k * 128:(k + 1) * 128, :])
            h32_prev.append(h3)

          ag_out_prev = None

          for s in range(STEPS):
             import concourse.mybir as _mb
             kt_u = 2 if s == 0 else KT

             # ---- stage 1 + q/r cascade: tq[m] = [fp8(t) | fp8(16(t-q))] ----
             tq = [None] * MT
             for mp in range(NC_CORES):
                 if "s1" not in ablate:
                     hc = p_hc.tile([128, KT, 2, 128], dt.float16, tag="hc")
                     blk = (h0t_p if s == 0 else ag_out_prev)[
                         D * mp:D * (mp + 1), :]
                     nc.sync.dma_start(
                         hc[:], blk.rearrange("(k p) mj -> p k mj", p=128))
                 for mloc in range(2):
                     m = 2 * mp + mloc
                     pt = p_mm.tile([128, D], dt.float32, tag="mm")
                     kt_s = 2 if s == 0 else KT
                     if "s1" in ablate:
                         nc.tensor.matmul(pt[:], wc_t[0][:, 0:128], wc_t[1][:],
                                          start=True, stop=True)
                     else:
                         for k in range(kt_s):
                             nc.tensor.matmul(pt[:], hc[:, k, mloc, :], wc_t[k][:],
                                              start=(k == 0), stop=(k == kt_s - 1))
                     tqm = p_t.tile([128, 2 * D], dt.float8e4, tag=f"tq{m}")
                     nc.scalar.copy(tqm[:, 0:D], pt[:])
                     tmp = p_tmp.tile([128, D], dt.float32, tag="tmp")
                     nc.vector.tensor_sub(tmp[:], pt[:], tqm[:, 0:D])
                     nc.scalar.activation(tqm[:, D:2 * D], tmp[:],
                                          mybir.ActivationFunctionType.Copy,
                                          scale=16.0)
                     tq[m] = tqm

             # ---- stage 2: DoubleRow kd-phases; RS per kd-pair ----
             rs_single = "rs_single" in ablate
             halves_cfg = ((0, 4),) if rs_single else ((0, 3), (1, 1))
             rs_ins, rs_outs = [], []
             for half, nk in halves_cfg:
                 ri = dram.tile([NC_CORES * nk * 128, SH], dt.float32,
                                tag=f"rs_in{half}", name=f"rs_in{half}")
                 ro = dram.tile([nk * 128, SH], dt.float32, tag=f"rs_out{half}",
                                name=f"rs_out{half}")
                 rs_ins.append(ri)
                 rs_outs.append(ro)
             for kd in range(KT):
                 accs = [p_mm.tile([128, D], dt.float32, tag="mm",
                                   name=f"acc{kd}_{q}") for q in range(4)]
                 if "s2" in ablate:
                     for q in range(4):
                         nc.tensor.matmul(accs[q][:], wc_t[0][:, 0:128],
                                          wc_t[1][:], start=True, stop=True)
                 else:
                     for m in range(MT):
                         lhs = tq[m][:].rearrange("p (o d) -> p o d", o=2)
                         rhs = at_t[m][:].rearrange("p (o n) -> p o n", o=2)
                         for q in range(4):
                             nc.tensor.matmul(
                                 accs[q][:], lhs[:, :, kd * 128:(kd + 1) * 128],
                                 rhs[:, :, q * D:(q + 1) * D],
                                 start=(m == 0), stop=(m == MT - 1),
                                 perf_mode=mybir.MatmulPerfMode.DoubleRow)
                 if rs_single:
                     half, nk, ki = 0, 4, kd
                 else:
                     half = 0 if kd < 3 else 1
                     nk = 3 if half == 0 else 1
                     ki = kd % 3
                 for q in range(4):
                     asb = p_asb.tile([128, D], dt.float32, tag="asb")
                     if q % 2 == 0:
                         nc.scalar.copy(asb[:], accs[q][:])
                     else:
                         nc.vector.tensor_copy(asb[:], accs[q][:])
                     eng = nc.sync if q % 2 == 0 else nc.scalar
                     for rr in range(2):
                         row0 = (2 * q + rr) * nk * 128 + ki * 128
                         eng.dma_start(rs_ins[half][row0:row0 + 128, :],
                                       asb[:, rr * SH:(rr + 1) * SH])
                 if (kd == 3) if rs_single else (kd in (2, 3)):
                     if "cc" in ablate or "rs" in ablate:
                         nc.sync.dma_start(rs_outs[half][:],
                                           rs_ins[half][0:nk * 128, :])
                     else:
                         nc.gpsimd.collective_compute(
                             "ReduceScatter", mybir.AluOpType.add,
                             replica_groups=RG,
                             ins=[rs_ins[half][:]], outs=[rs_outs[half][:]])

             # ---- GRU ----
             # z/r U-parts (local h -> overlap RS latency)
             pz = [p_mm.tile([128, SH], dt.float32, tag="mm", name=f"pz{f}")
                   for f in range(KT)]
             pr = [p_mm.tile([128, SH], dt.float32, tag="mm", name=f"pr{f}")
                   for f in range(KT)]
             if "gru" in ablate:
                 for f in range(KT):
                     nc.tensor.matmul(pz[f][:], wc_t[0][:, 0:128],
                                      wc_t[1][:, 0:SH], start=True, stop=True)
                     nc.tensor.matmul(pr[f][:], wc_t[0][:, 0:128],
                                      wc_t[1][:, 0:SH], start=True, stop=True)
             else:
                 for pg_l, uidx in ((pz, 1), (pr, 3)):
                     Uq = gw_res[uidx]
                     for f in range(KT):
                         for k in range(kt_u):
                             nc.tensor.matmul(pg_l[f][:],
                                              Uq[:, k, f * 128:(f + 1) * 128],
                                              hsh_prev[k][:],
                                              start=(k == 0), stop=False)

             # aT tiles: RS outputs + bias_a, fp16
             aT = []
             for k in range(KT):
                 an = p_sm.tile([128, SH], dt.float32, tag=f"an{k}")
                 src_half = 0 if (rs_single or k < 3) else 1
                 r0 = (k if rs_single else (k % 3)) * 128
                 nc.sync.dma_start(an[:], rs_outs[src_half][r0:r0 + 128, :])
                 a_k = p_sm.tile([128, SH], dt.float32r, tag=f"aT{k}")
                 nc.vector.tensor_add(a_k[:], an[:], ba_t[k][:])
                 aT.append(a_k)

             if "gru" not in ablate:
                 # W-parts k-outer: k<3 consume RS-A, k=3 consumes RS-B
                 for k in range(KT):
                     for pg_l, widx in ((pz, 0), (pr, 2)):
                         Wq = gw_res[widx]
                         for f in range(KT):
                             nc.tensor.matmul(pg_l[f][:],
                                              Wq[:, k, f * 128:(f + 1) * 128],
                                              aT[k][:],
                                              start=False, stop=(k == KT - 1))
             z_t, r_t = [], []
             for outs, pg_l, nm, fn in (
                     (z_t, pz, "z", _mb.ActivationFunctionType.Sigmoid),
                     (r_t, pr, "r", _mb.ActivationFunctionType.Sigmoid)):
                 for f in range(KT):
                     og = p_sm.tile([128, SH], dt.float32, tag=f"g{nm}{f}",
                                    name=f"g{nm}{f}")
                     nc.scalar.activation(og[:], pg_l[f][:], fn,
                                          bias=bias_tiles[(nm, f)][:])
                     outs.append(og)
             rh = []
             for k in range(KT):
                 rhk = p_sm.tile([128, SH], dt.float32r, tag=f"rh{k}")
                 nc.vector.tensor_mul(rhk[:], r_t[k][:], h32_prev[k][:])
                 rh.append(rhk)
             ht_t = []
             ph = [p_mm.tile([128, SH], dt.float32, tag="mm", name=f"ph{f}")
                   for f in range(KT)]
             if "gru" in ablate:
                 for f in range(KT):
                     nc.tensor.matmul(ph[f][:], wc_t[0][:, 0:128],
                                      wc_t[1][:, 0:SH], start=True, stop=True)
             else:
                 Wq, Uq = gw_res[4], gw_res[5]
                 for f in range(KT):
                     for k in range(kt_u):
                         nc.tensor.matmul(ph[f][:],
                                          Uq[:, k, f * 128:(f + 1) * 128],
                                          rh[k][:], start=(k == 0), stop=False)
                     for k in range(KT):
                         nc.tensor.matmul(ph[f][:],
                                          Wq[:, k, f * 128:(f + 1) * 128],
                                          aT[k][:], start=False,
                                          stop=(k == KT - 1))
             for f in range(KT):
                 og = p_sm.tile([128, SH], dt.float32, tag=f"gh{f}",
                                name=f"gh{f}")
                 nc.scalar.activation(og[:], ph[f][:],
                                      _mb.ActivationFunctionType.Tanh,
                                      bias=bias_tiles[("h", f)][:])
                 ht_t.append(og)

             # ---- h' = h + z * (ht - h) ----
             hsh_new, h32_new = [], []
             last = (s == STEPS - 1)
             if not last:
                 ag_in = dram.tile([D, SH], dt.float16, tag="ag_in")
             for k in range(KT):
                 s1 = p_sm.tile([128, SH], dt.float32, tag="gsA")
                 nc.vector.tensor_sub(s1[:], ht_t[k][:], h32_prev[k][:])
                 s2 = p_sm.tile([128, SH], dt.float32, tag="gsB")
                 nc.vector.tensor_mul(s2[:], z_t[k][:], s1[:])
                 h3 = p_h.tile([128, SH], dt.float32, tag=f"h32{k}")
                 nc.vector.tensor_add(h3[:], h32_prev[k][:], s2[:])
                 h32_new.append(h3)
                 if last:
                     nc.sync.dma_start(out_p[k * 128:(k + 1) * 128, :], h3[:])
                 else:
                     hr = p_h.tile([128, SH], dt.float32r, tag=f"hnr{k}")
                     nc.vector.tensor_copy(hr[:], h3[:])
                     hsh_new.append(hr)
                     h16 = p_sm.tile([128, SH], dt.float16, tag=f"h16{k}",
                                     name=f"h16{k}")
                     nc.scalar.copy(h16[:], h3[:])
                     nc.sync.dma_start(ag_in[k * 128:(k + 1) * 128, :], h16[:])

             if not last:
                 ag_out = dram.tile([NC_CORES * D, SH], dt.float16, tag="ag_out",
                                    addr_space="Shared")
                 if "cc" in ablate or "ag" in ablate:
                     nc.sync.dma_start(ag_out[0:D, :], ag_in[:])
                 else:
                     nc.gpsimd.collective_compute(
                         "AllGather", mybir.AluOpType.bypass, replica_groups=RG,
                         ins=[ag_in[:]], outs=[ag_out[:]])
                 ag_out_prev = ag_out
                 hsh_prev, h32_prev = hsh_new, h32_new

    nc.finalize()
    return nc


def build3(repeats=1, ablate=()):
    """v3: contract-dim (j) sharding — no AllGather, one fp16 RS per step.

    Core c owns nodes shard_c = [256c, 256c+256). Per step, on core c:
      stage1: t_e[shard_c] = h_shard @ W_e  for ALL 8 edge types  (fp32r)
              -> tq[m] fp8 hi|lo pairs, m = 2e+j2 (16 tiles of 128 j)
      stage2: partial aT[d, n] = sum_m tq[m].T (x) A_sel[m]  (DoubleRow,
              rhs slot dim broadcast stride-0: hi pairs A, lo pairs A)
      RS:     fp16 rank-blocked ReduceScatter of partial aT, split
              (kd0 | kd1-3) so the first RS overlaps stage2's tail
      GRU:    U-parts (local h, fp32r) overlap RS flight; W-parts fp16
              consume aT as halves arrive; h'^T stays local — no AG.
    """
    import concourse.bacc as bacc
    import concourse.mybir as mybir
    import concourse.tile as tile

    dt = mybir.dt
    nc = bacc.Bacc()
    at8_p = nc.declare_dram_parameter("at8", [N // 2, 2 * N], dt.float8e4,
                                      isOutput=False)
    h0s_p = nc.declare_dram_parameter("h0s", [D, SH], dt.float32r,
                                  isOutput=False)
    wpr_p = nc.declare_dram_parameter("wpr", [2 * E_TYPES, D, D], dt.float32r,
                                      isOutput=False)
    gwu_p = nc.declare_dram_parameter("gwu", [3, D, D], dt.float32r,
                                      isOutput=False)
    gww_p = nc.declare_dram_parameter("gww", [3, D, D], dt.float32r,
                                      isOutput=False)
    ba_p = nc.declare_dram_parameter("ba", [D, SH], dt.float32, isOutput=False)
    bz_p = nc.declare_dram_parameter("bzc", [D, 1], dt.float32, isOutput=False)
    br_p = nc.declare_dram_parameter("brc", [D, 1], dt.float32, isOutput=False)
    bh_p = nc.declare_dram_parameter("bhc", [D, 1], dt.float32, isOutput=False)
    out_p = nc.declare_dram_parameter("out", [D, SH], dt.float32, isOutput=True)
    RG = [list(range(NC_CORES))]

    from contextlib import ExitStack
    with tile.TileContext(nc) as tc, ExitStack() as stk:
        res = stk.enter_context(tc.tile_pool(name="res", bufs=1))
        p_mm = stk.enter_context(tc.tile_pool(name="pmm", bufs=8, space="PSUM"))
        p_t = stk.enter_context(tc.tile_pool(name="pt", bufs=1))
        p_asb = stk.enter_context(tc.tile_pool(name="pasb", bufs=2))
        p_sm = stk.enter_context(tc.tile_pool(name="psm", bufs=1))
        p_h = stk.enter_context(tc.tile_pool(name="ph", bufs=2))
        dram = stk.enter_context(tc.tile_pool(name="dram", bufs=2, space="DRAM"))

        # ---- resident setup ----
        bias_tiles = {}
        for nm, par in (("z", bz_p), ("r", br_p), ("h", bh_p)):
            for f in range(KT):
                bt = res.tile([128, 1], dt.float32, tag=f"b{nm}{f}")
                nc.sync.dma_start(bt[:], par[f * 128:(f + 1) * 128, :])
                bias_tiles[(nm, f)] = bt

        ba_t = []
        for k in range(KT):
            b = res.tile([128, SH], dt.float32, tag=f"ba{k}")
            nc.sync.dma_start(b[:], ba_p[k * 128:(k + 1) * 128, :])
            ba_t.append(b)

        at_t = []
        for mp in range(MT // 2):
            a = res.tile([128, 2, N], dt.float8e4, tag=f"at{mp}")
            nc.scalar.dma_start(
                a[:], at8_p[mp * 128:(mp + 1) * 128, :].rearrange(
                    "p (o n) -> p o n", o=2))
            at_t.append(a)

        wpr_t = []
        for e in range(2 * E_TYPES):
            w = res.tile([128, KT, D], dt.float32r, tag=f"wpr{e}")
            nc.scalar.dma_start(w[:], wpr_p[e].rearrange("(k p) f -> p k f",
                                                         p=128))
            wpr_t.append(w)

        gwu_t, gww_t = [], []
        for g in range(3):
            wu = res.tile([128, KT, D], dt.float32r, tag=f"gwu{g}")
            nc.scalar.dma_start(wu[:], gwu_p[g].rearrange("(k p) f -> p k f",
                                                          p=128))
            gwu_t.append(wu)
            ww = res.tile([128, KT, D], dt.float32r, tag=f"gww{g}")
            nc.scalar.dma_start(ww[:], gww_p[g].rearrange("(k p) f -> p k f",
                                                          p=128))
            gww_t.append(ww)

        for rep in range(repeats):
          h32_prev = []
          for k in range(KT):
            h3 = p_h.tile([128, SH], dt.float32r, tag=f"h32{k}")
            nc.sync.dma_start(h3[:], h0s_p[k * 128:(k + 1) * 128, :])
            h32_prev.append(h3)

          for s in range(STEPS):
             import concourse.mybir as _mb
             kt_h = 2 if s == 0 else KT   # h^T rows >=256 are zero at step 0

             # ---- stage 1 + fp8 hi|lo quant ----
             # k-inner-e loop shares each h-slice LDWEIGHTS across 8 MMs
             tqh = [p_t.tile([128, 2, D], dt.float8e4, tag=f"tqh{i}",
                             name=f"tqh{i}") for i in range(8)]
             tql = [p_t.tile([128, 2, D], dt.float8e4, tag=f"tql{i}",
                             name=f"tql{i}") for i in range(8)]
             for j2 in range(2):
                 pts = [p_mm.tile([128, D], dt.float32, tag="mm",
                                  name=f"pt{j2}_{e}")
                        for e in range(2 * E_TYPES)]
                 if "s1" in ablate:
                     for e in range(2 * E_TYPES):
                         nc.tensor.matmul(pts[e][:], wpr_t[0][:, 0, 0:128],
                                          wpr_t[0][:, 1, :], start=True,
                                          stop=True)
                 else:
                     for k in range(kt_h):
                         for e in range(2 * E_TYPES):
                             nc.tensor.matmul(
                                 pts[e][:],
                                 h32_prev[k][:, j2 * 128:(j2 + 1) * 128],
                                 wpr_t[e][:, k, :],
                                 start=(k == 0), stop=(k == kt_h - 1))
                 for e in range(2 * E_TYPES):
                     m = 2 * e + j2
                     hi = tqh[m % 8][:, m // 8, :]
                     nc.scalar.copy(hi, pts[e][:])
                     nc.vector.tensor_sub(tql[m % 8][:, m // 8, :], pts[e][:],
                                          hi)

             # ---- stage 2 (DoubleRow) + per-kd fp16 ReduceScatter ----
             # one small RS per kd phase: each pipelines behind the next
             # phase's matmuls; only the last phase's RS is exposed (~5us)
             rs_ins, rs_outs = [], []
             for half in range(KT):
                 ri = dram.tile([NC_CORES * 128, SH], dt.float16,
                                tag=f"rs_in{half}", name=f"rs_in{half}")
                 ro = dram.tile([128, SH], dt.float16,
                                tag=f"rs_out{half}", name=f"rs_out{half}")
                 rs_ins.append(ri)
                 rs_outs.append(ro)
             for kd in range(KT):
                 half = kd
                 nk = 1
                 ki = 0
                 accs = [p_mm.tile([128, D], dt.float32, tag="mm",
                                   name=f"acc{kd}_{q}") for q in range(4)]
                 if "s2" in ablate:
                     for q in range(4):
                         nc.tensor.matmul(accs[q][:], wpr_t[0][:, 0, 0:128],
                                          wpr_t[0][:, 1, :], start=True,
                                          stop=True)
                 else:
                     for mp in range(MT // 2):
                         for hl, tqx in ((0, tqh), (1, tql)):
                             lhs = tqx[mp][:, :, kd * 128:(kd + 1) * 128]
                             for q in range(4):
                                 nc.tensor.matmul(
                                     accs[q][:], lhs,
                                     at_t[mp][:, :, q * D:(q + 1) * D],
                                     start=(mp == 0 and hl == 0),
                                     stop=(mp == MT // 2 - 1 and hl == 1),
                                     perf_mode=mybir.MatmulPerfMode.DoubleRow)
                 asb = p_asb.tile([128, N], dt.float16, tag="asb")
                 for q in range(4):
                     if q % 2 == 0:
                         nc.scalar.copy(asb[:, q * D:(q + 1) * D], accs[q][:])
                     else:
                         nc.vector.tensor_copy(asb[:, q * D:(q + 1) * D],
                                               accs[q][:])
                 for r in range(NC_CORES):
                     eng = nc.sync if r % 2 == 0 else nc.scalar
                     eng.dma_start(
                         rs_ins[half][r * nk * 128 + ki * 128:
                                      r * nk * 128 + ki * 128 + 128, :],
                         asb[:, r * SH:(r + 1) * SH])
                 if "cc" in ablate or "rs" in ablate:
                     nc.sync.dma_start(rs_outs[half][:],
                                       rs_ins[half][0:nk * 128, :])
                 else:
                     nc.gpsimd.collective_compute(
                         "ReduceScatter", mybir.AluOpType.add,
                         replica_groups=RG,
                         ins=[rs_ins[half][:]], outs=[rs_outs[half][:]])

             # ---- GRU ----
             # U-parts (depend only on local h) overlap the RS flight
             pz = [p_mm.tile([128, SH], dt.float32, tag="mm", name=f"pz{f}")
                   for f in range(KT)]
             pr = [p_mm.tile([128, SH], dt.float32, tag="mm", name=f"pr{f}")
                   for f in range(KT)]
             if "gru" in ablate:
                 for f in range(KT):
                     nc.tensor.matmul(pz[f][:], wpr_t[0][:, 0, 0:128],
                                      wpr_t[0][:, 1, 0:SH], start=True,
                                      stop=True)
                     nc.tensor.matmul(pr[f][:], wpr_t[0][:, 0, 0:128],
                                      wpr_t[0][:, 1, 0:SH], start=True,
                                      stop=True)
             else:
                 for pg_l, uidx in ((pz, 0), (pr, 1)):
                     Uq = gwu_t[uidx]
                     for f in range(KT):
                         for k in range(kt_h):
                             nc.tensor.matmul(
                                 pg_l[f][:], Uq[:, k, f * 128:(f + 1) * 128],
                                 h32_prev[k][:],
                                 start=(k == 0), stop=False)

             # aT tiles (fp16): kd0 from RS half A, kd1-3 from half B
             aT = []
             for k in range(KT):
                 an = p_sm.tile([128, SH], dt.float16, tag=f"an{k}")
                 nc.sync.dma_start(an[:], rs_outs[k][:])
                 a_k = p_sm.tile([128, SH], dt.float32r, tag=f"aT{k}")
                 nc.vector.tensor_add(a_k[:], an[:], ba_t[k][:])
                 aT.append(a_k)

             if "gru" not in ablate:
                 for k in range(KT):
                     for pg_l, widx in ((pz, 0), (pr, 1)):
                         Wq = gww_t[widx]
                         for f in range(KT):
                             nc.tensor.matmul(
                                 pg_l[f][:], Wq[:, k, f * 128:(f + 1) * 128],
                                 aT[k][:], start=False, stop=(k == KT - 1))
             z_t, r_t = [], []
             for outs, pg_l, nm, fn in (
                     (z_t, pz, "z", _mb.ActivationFunctionType.Sigmoid),
                     (r_t, pr, "r", _mb.ActivationFunctionType.Sigmoid)):
                 for f in range(KT):
                     og = p_sm.tile([128, SH], dt.float32, tag=f"g{nm}{f}",
                                    name=f"g{nm}{f}")
                     nc.scalar.activation(og[:], pg_l[f][:], fn,
                                          bias=bias_tiles[(nm, f)][:])
                     outs.append(og)
             rh = []
             for k in range(KT):
                 rhk = p_sm.tile([128, SH], dt.float32r, tag=f"rh{k}")
                 nc.vector.tensor_mul(rhk[:], r_t[k][:], h32_prev[k][:])
                 rh.append(rhk)
             ht_t = []
             ph = [p_mm.tile([128, SH], dt.float32, tag="mm", name=f"phh{f}")
                   for f in range(KT)]
             if "gru" in ablate:
                 for f in range(KT):
                     nc.tensor.matmul(ph[f][:], wpr_t[0][:, 0, 0:128],
                                      wpr_t[0][:, 1, 0:SH], start=True,
                                      stop=True)
             else:
                 Uq, Wq = gwu_t[2], gww_t[2]
                 for f in range(KT):
                     for k in range(kt_h):
                         nc.tensor.matmul(
                             ph[f][:], Uq[:, k, f * 128:(f + 1) * 128],
                             rh[k][:],
                             start=(k == 0), stop=False)
                     for k in range(KT):
                         nc.tensor.matmul(
                             ph[f][:], Wq[:, k, f * 128:(f + 1) * 128],
                             aT[k][:], start=False, stop=(k == KT - 1))
             for f in range(KT):
                 og = p_sm.tile([128, SH], dt.float32, tag=f"gh{f}",
                                name=f"gh{f}")
                 nc.scalar.activation(og[:], ph[f][:],
                                      _mb.ActivationFunctionType.Tanh,
                                      bias=bias_tiles[("h", f)][:])
                 ht_t.append(og)

             # ---- h' = h + z * (ht - h) ----
             h32_new = []
             last = (s == STEPS - 1)
             for k in range(KT):
                 s1t = p_sm.tile([128, SH], dt.float32, tag="gsA")
                 nc.vector.tensor_sub(s1t[:], ht_t[k][:], h32_prev[k][:])
                 s2t = p_sm.tile([128, SH], dt.float32, tag="gsB")
                 nc.vector.tensor_mul(s2t[:], z_t[k][:], s1t[:])
                 h3 = p_h.tile([128, SH], dt.float32r, tag=f"h32{k}")
                 nc.vector.tensor_add(h3[:], h32_prev[k][:], s2t[:])
                 h32_new.append(h3)
                 if last:
                     nc.sync.dma_start(out_p[k * 128:(k + 1) * 128, :],
                                       h3[:].bitcast(dt.float32))
             if not last:
                 h32_prev = h32_new

    nc.finalize()
    return nc


def prepare_in_maps3(adjacency, annotations, W_prop, b_prop, Wz, Uz, bz,
                     Wr, Ur, br, Wh, Uh, bh):
    A = np.asarray(adjacency, np.float32)
    ann = np.asarray(annotations, np.float32)
    W_prop = np.asarray(W_prop, np.float32)
    b_prop = np.asarray(b_prop, np.float32)
    gwu = _q12(np.stack([np.asarray(x, np.float32) for x in (Uz, Ur, Uh)]))
    gww = _q12(np.stack([np.asarray(x, np.float32)
                     for x in (Wz, Wr, Wh)]) / 16.0)
    wpr = _q12(16.0 * W_prop)
    bz = np.asarray(bz, np.float32).reshape(D, 1)
    br = np.asarray(br, np.float32).reshape(D, 1)
    bh = np.asarray(bh, np.float32).reshape(D, 1)

    h0 = np.zeros((N, D), np.float32)
    h0[:, :ann.shape[1]] = ann
    h0t = np.ascontiguousarray(h0.T)                    # [D, N]

    # bias_a[n, :] = sum_e deg_e(n) * b_e ; transposed shard [D, SH]
    deg = A.reshape(N, 2 * E_TYPES, N).sum(axis=2)      # [N, 2E]
    bias_aT = np.ascontiguousarray(16.0 * (deg @ b_prop).T)    # [D, N]

    Ar = A.reshape(N, 2 * E_TYPES, N)                   # [n, e, j]
    in_maps = []
    for c in range(NC_CORES):
        sel = Ar[:, :, c * SH:(c + 1) * SH]             # [n, e, jj]
        asel = sel.transpose(1, 2, 0).reshape(N, N)     # rows (e,jj), cols n
        # slot-paired layout: row mp*128+p, cols [blk(mp) | blk(mp+8)]
        at8 = np.concatenate(
            [asel[:N // 2].reshape(8, 128, N), asel[N // 2:].reshape(8, 128, N)],
            axis=2).reshape(N // 2, 2 * N).astype(ml_dtypes.float8_e4m3)
        at8 = np.ascontiguousarray(at8)
        in_maps.append({
            "at8": at8,
            "h0s": _q12(np.ascontiguousarray(h0t[:, c * SH:(c + 1) * SH])),
            "wpr": wpr,
            "gwu": gwu,
            "gww": gww,
            "ba": np.ascontiguousarray(bias_aT[:, c * SH:(c + 1) * SH]),
            "bzc": bz, "brc": br, "bhc": bh,
        })
    return in_maps


def prepare_in_maps2(adjacency, annotations, W_prop, b_prop, Wz, Uz, bz,
                     Wr, Ur, br, Wh, Uh, bh):
    A = np.asarray(adjacency, np.float32)
    ann = np.asarray(annotations, np.float32)
    W_prop = np.asarray(W_prop, np.float32)
    b_prop = np.asarray(b_prop, np.float32)
    gw_all = _q12(np.stack([np.asarray(x, np.float32)
                            for x in (Wz, Uz, Wr, Ur, Wh, Uh)]))
    bz = np.asarray(bz, np.float32).reshape(D, 1)
    br = np.asarray(br, np.float32).reshape(D, 1)
    bh = np.asarray(bh, np.float32).reshape(D, 1)

    h0 = np.zeros((N, D), np.float32)
    h0[:, :ann.shape[1]] = ann
    h0t = np.ascontiguousarray(h0.T)           # [D, N] fp32
    h0t16 = h0t.astype(np.float16)
    A_T = np.ascontiguousarray(A.T)            # [2E*N, N]

    # bias_a[n, :] = sum_e deg_e(n) * b_e ; transposed shard [D, SH]
    deg = A.reshape(N, 2 * E_TYPES, N).sum(axis=2)      # [N, 2E]
    bias_a = deg @ b_prop                               # [N, D]
    bias_aT = np.ascontiguousarray(bias_a.T)            # [D, N]

    # contiguous shards: core c owns nodes 256c..256c+255
    h0t_ag = np.ascontiguousarray(h0t16)  # same layout: [D, N] -> per-core
    # AG layout: [NC*D, SH]: block mp = core mp's [D, SH]
    h0t_ag = np.concatenate(
        [h0t16[:, c * SH:(c + 1) * SH] for c in range(NC_CORES)], axis=0)

    in_maps = []
    for c in range(NC_CORES):
        at_c = A_T[c * N:(c + 1) * N, :]               # [N j, N n] 0/1
        at8 = at_c.astype(ml_dtypes.float8_e4m3)
        at8_lo = (at_c / 16.0).astype(ml_dtypes.float8_e4m3)
        at2 = np.concatenate([at8, at8_lo], axis=1)    # [N, 2N] slot-major
        in_maps.append({
            "at2": np.ascontiguousarray(at2),
            "h0t": np.ascontiguousarray(h0t_ag),
            "h0sr": _q12(np.ascontiguousarray(h0t[:, c * SH:(c + 1) * SH])),
            "h0s": _q12(np.ascontiguousarray(h0t[:, c * SH:(c + 1) * SH])),
            "wc": W_prop[c].astype(np.float16),
            "gw": gw_all,
            "ba": np.ascontiguousarray(bias_aT[:, c * SH:(c + 1) * SH]),
            "bzc": bz, "brc": br, "bhc": bh,
        })
    return in_maps


E_TYPES = 4
_BUILT = None
TRACE = False
V2 = True
V3 = True
LAST_RESULT = None


_BUILT_R = {}


def _get_built(repeats=1, ablate=()):
    global _BUILT
    bf = build3 if V3 else (build2 if V2 else build)
    key = (V3, V2, repeats, tuple(ablate))
    if key != (V3, V2, 1, ()):
        if key not in _BUILT_R:
            _BUILT_R[key] = bf(repeats, ablate)
        return _BUILT_R[key]
    if _BUILT is None:
        _BUILT = bf()
    return _BUILT


def prepare_in_maps(adjacency, annotations, W_prop, b_prop, Wz, Uz, bz,
                    Wr, Ur, br, Wh, Uh, bh):
    A = np.asarray(adjacency, np.float32)
    ann = np.asarray(annotations, np.float32)
    W_prop = np.asarray(W_prop, np.float32)
    b_prop = np.asarray(b_prop, np.float32)
    gw_all = _q12(np.stack([np.asarray(x, np.float32)
                            for x in (Wz, Uz, Wr, Ur, Wh, Uh)]))
    bz = np.asarray(bz, np.float32).reshape(D, 1)
    br = np.asarray(br, np.float32).reshape(D, 1)
    bh = np.asarray(bh, np.float32).reshape(D, 1)

    h0 = np.zeros((N, D), np.float32)
    h0[:, :ann.shape[1]] = ann
    h0t = np.ascontiguousarray(h0.T)           # [D, N] fp32
    h0t_r = _q12(h0t)
    A_T = np.ascontiguousarray(A.T)            # [2E*N, N]

    # shard layout: core c owns node blocks {128c..128c+127, 1024+128c..+127}
    shard_cols = [np.r_[128 * c:128 * c + 128, 1024 + 128 * c:1024 + 128 * c + 128]
                  for c in range(NC_CORES)]
    h0t_ag = np.ascontiguousarray(np.concatenate(
        [h0t_r[:, shard_cols[c]] for c in range(NC_CORES)], axis=0))

    in_maps = []
    for c in range(NC_CORES):
        in_maps.append({
            "at": np.ascontiguousarray(
                A_T[c * N:(c + 1) * N, :]).astype(np.uint8),
            "h0t": h0t_ag,
            "h0sr": np.ascontiguousarray(h0t_r[:, shard_cols[c]]),
            "h0s": np.ascontiguousarray(h0t[:, shard_cols[c]]),
            "wc": _q12(W_prop[c]),
            "gw": gw_all,
            "bpc": np.ascontiguousarray(b_prop[c].reshape(1, D)),
            "bzc": bz, "brc": br, "bhc": bh,
        })

    return in_maps


def kernel(**inputs):
    from concourse.bass_utils import run_bass_kernel_spmd

    prep = (prepare_in_maps3 if V3 else
            (prepare_in_maps2 if V2 else prepare_in_maps))
    in_maps = prep(
        **{k: inputs[k] for k in ("adjacency", "annotations", "W_prop", "b_prop",
                                  "Wz", "Uz", "bz", "Wr", "Ur", "br",
                                  "Wh", "Uh", "bh")})
    nc = _get_built()
    res = run_bass_kernel_spmd(nc, in_maps, list(range(NC_CORES)), trace=TRACE)
    global LAST_RESULT
    LAST_RESULT = res
    h = np.empty((N, D), np.float32)
    for c in range(NC_CORES):
        sh = res.results[c]["out"].T           # [SH, D] rows in shard order
        if V3 or V2:
            h[SH * c:SH * (c + 1)] = sh
        else:
            h[128 * c:128 * c + 128] = sh[:128]
            h[1024 + 128 * c:1024 + 128 * c + 128] = sh[128:]
    return h

